# revision 4
# baseline (speedup 1.0000x reference)
"""Trainium2 Bass kernel for the FFT-block (attention + conv FFN) problem.

Sharding: data-parallel over batch. B=16 items across 8 cores -> 2 items/core.
Each core runs the full block for its items; no collectives.

Layout strategy per item:
  - attention computed via scores^T = K Q^T (so softmax sums land on the
    partition axis and are folded into the ctx matmul via a ones-column in V)
  - convs are 9 shifted matmuls over transposed activations hT [D, S_pad],
    fp32r (tf32-like) matmuls with fp32 PSUM accumulation everywhere.
"""
import sys, types
import numpy as np

B, S, D = 16, 1024, 512
H, DK = 8, 64
CD, KS = 2048, 9
EPS = 1e-5
NCORES = 8
NIT = B // NCORES          # items per core
NDC = D // 128             # 4 d-chunks
NSC = S // 128             # 8 s-chunks
NCOL = S // 512            # 2 s-cols
NCD = CD // 128            # 16 cd-chunks


def _install_ntff_hook():
    try:
        from antenv.axon_hooks import get_axon_ntff_profile_hook  # noqa
        return
    except ImportError:
        pass
    try:
        from trn_agent_boot.trn_boot import _ntff_profile_via_ctypes
        mod = types.ModuleType('antenv.axon_hooks')
        hook = _ntff_profile_via_ctypes('/opt/axon/libaxon_pjrt.so')
        mod.get_axon_ntff_profile_hook = lambda: hook
        sys.modules['antenv.axon_hooks'] = mod
    except Exception:
        pass


_BUILT = None


def _build():
    global _BUILT
    if _BUILT is not None:
        return _BUILT
    _install_ntff_hook()
    import concourse.bacc as bacc
    import concourse.mybir as mybir
    from concourse import tile
    from concourse.masks import make_identity

    F32 = mybir.dt.float32
    F32R = mybir.dt.float32r
    AF = mybir.ActivationFunctionType
    ALU = mybir.AluOpType

    nc = bacc.Bacc("TRN2", target_bir_lowering=False, debug=False,
                   num_devices=NCORES)

    # ---- DRAM I/O (per core) ----
    d_xT = nc.dram_tensor("xT", [NIT, NDC, 128, S], F32, kind="ExternalInput")
    d_xp = nc.dram_tensor("xp", [NIT, NSC, 128, D], F32, kind="ExternalInput")
    d_wqk = nc.dram_tensor("wqk", [2, 4, 128, 512], F32, kind="ExternalInput")
    d_bqk = nc.dram_tensor("bqk", [128, 8], F32, kind="ExternalInput")
    d_wv = nc.dram_tensor("wv", [NDC, 128, 520], F32, kind="ExternalInput")
    d_bvrow = nc.dram_tensor("bvrow", [128, 520], F32, kind="ExternalInput")
    d_wo = nc.dram_tensor("wo", [4, 128, 512], F32, kind="ExternalInput")
    d_w1 = nc.dram_tensor("w1", [NCD, NDC, 128, KS * 128], F32, kind="ExternalInput")
    d_w2 = nc.dram_tensor("w2", [NCD, 128, KS * 512], F32, kind="ExternalInput")
    d_bc1s = nc.dram_tensor("bc1s", [128, NCD], F32, kind="ExternalInput")
    d_gb = nc.dram_tensor("gb", [5, 128, 512], F32, kind="ExternalInput")
    d_cones = nc.dram_tensor("cones", [128, 128], F32, kind="ExternalInput")
    d_czero = nc.dram_tensor("czero", [128, 8], F32, kind="ExternalInput")
    d_y = nc.dram_tensor("y", [NIT, NSC, 128, D], F32, kind="ExternalOutput")

    G1, B1, G2, B2, BC2 = range(5)

    with tile.TileContext(nc) as tc:
        from contextlib import ExitStack
        est = ExitStack()
        with est:
            cp = est.enter_context(tc.tile_pool(name="const", bufs=1))
            hp = est.enter_context(tc.tile_pool(name="hTpool", bufs=1))
            dp = est.enter_context(tc.tile_pool(name="dramp", bufs=1, space="DRAM"))
            h_dram = [[dp.tile([128, D], F32, tag=f"hd{it}_{sc}", name=f"hd{it}_{sc}")
                       for sc in range(NSC)] for it in range(NIT)]

            t_bqk = cp.tile([128, 8], F32, tag="bqk")
            nc.sync.dma_start(t_bqk[:], d_bqk[:])
            t_gb = []
            for i in range(5):
                t = cp.tile([128, 512], F32, tag=f"gb{i}")
                nc.sync.dma_start(t[:], d_gb[i])
                t_gb.append(t)
            t_bc1s = cp.tile([128, NCD], F32, tag="bc1s")
            nc.sync.dma_start(t_bc1s[:], d_bc1s[:])
            t_ident = cp.tile([128, 128], F32, tag="ident")
            make_identity(nc, t_ident[:])
            t_cones = cp.tile([128, 128], F32R, tag="cones")
            nc.sync.dma_start(t_cones[:], d_cones[:].bitcast(F32R))
            t_czero = cp.tile([128, 8], F32R, tag="czero")
            nc.sync.dma_start(t_czero[:], d_czero[:].bitcast(F32R))
            t_eps = cp.tile([128, 1], F32, tag="eps")
            nc.vector.memset(t_eps[:], EPS)

            # persistent hT tiles  [d-chunk partitions, padded s]
            hT = [[hp.tile([128, S + 8], F32R, tag=f"ht{it}_{dc}", name=f"ht{it}_{dc}")
                   for dc in range(NDC)] for it in range(NIT)]

            # ---------------- Phase A: attention + LN1 + transpose ----------
            with ExitStack() as pa:
                pla = pa.enter_context(tc.tile_pool(name="pa", bufs=1))
                psa = pa.enter_context(tc.tile_pool(name="psa", bufs=6, space="PSUM"))

                t_wv = []
                for dc in range(NDC):
                    t = pla.tile([128, 520], F32R, tag=f"wv{dc}")
                    nc.sync.dma_start(t[:], d_wv[dc].bitcast(F32R))
                    t_wv.append(t)
                t_bvrow = pla.tile([128, 520], F32R, tag="bvrow")
                nc.sync.dma_start(t_bvrow[:], d_bvrow[:].bitcast(F32R))
                t_wo = []
                for c in range(4):
                    t = pla.tile([128, 512], F32R, tag=f"wo{c}")
                    nc.sync.dma_start(t[:], d_wo[c].bitcast(F32R))
                    t_wo.append(t)

                for it in range(NIT):
                    # load xT
                    xt = []
                    for dc in range(NDC):
                        t = pla.tile([128, S], F32R, tag=f"xt{dc}")
                        nc.sync.dma_start(t[:], d_xT[it, dc].bitcast(F32R))
                        xt.append(t)

                    # V projection (with bias row and per-head ones column)
                    vst = []
                    for tc_i in range(NSC):
                        vt = pla.tile([128, 520], F32R, tag=f"vst{tc_i}")
                        for half in range(2):
                            colo = half * 260
                            pv = psa.tile([128, 260], F32, tag="p", bufs=2)
                            for dc in range(NDC):
                                nc.tensor.matmul(
                                    pv[:], xt[dc][:, tc_i * 128:(tc_i + 1) * 128],
                                    t_wv[dc][:, colo:colo + 260],
                                    start=(dc == 0), stop=False)
                            nc.tensor.matmul(
                                pv[:], t_cones[0:1, 0:128],
                                t_bvrow[0:1, colo:colo + 260],
                                start=False, stop=True)
                            nc.vector.tensor_copy(vt[:, colo:colo + 260], pv[:])
                        vst.append(vt)

                    # ctxT output tiles (head pair c -> rows, s -> cols)
                    ctxT = [pla.tile([128, S], F32R, tag=f"ct{c}", name=f"ct{c}") for c in range(4)]

                    for pair in range(4):
                        qk = []
                        for proj in range(2):
                            wt = pla.tile([128, 512], F32R, tag=f"wqk{proj}", bufs=2)
                            nc.sync.dma_start(wt[:], d_wqk[proj, pair].bitcast(F32R))
                            qt = pla.tile([128, S], F32R, tag=f"qk{proj}")
                            for scol in range(NCOL):
                                pq = psa.tile([128, 512], F32, tag="p", bufs=2)
                                for dc in range(NDC):
                                    nc.tensor.matmul(
                                        pq[:], wt[:, dc * 128:(dc + 1) * 128],
                                        xt[dc][:, scol * 512:(scol + 1) * 512],
                                        start=(dc == 0), stop=(dc == NDC - 1))
                                nc.vector.tensor_scalar_add(
                                    qt[:, scol * 512:(scol + 1) * 512], pq[:],
                                    t_bqk[:, proj * 4 + pair:proj * 4 + pair + 1])
                            qk.append(qt)
                        qT, kT = qk

                        for sub in range(2):
                            h = 2 * pair + sub
                            hr = slice(sub * 64, sub * 64 + 64)
                            for scol in range(NCOL):
                                so = scol * 512
                                # P_exp = exp(scores^T / 8), per t-chunk pair
                                pex = []
                                for ti in range(0, NSC, 2):
                                    pp = psa.tile([128, 1024], F32, tag="p2", bufs=2)
                                    for j in range(2):
                                        nc.tensor.matmul(
                                            pp[:, j * 512:(j + 1) * 512],
                                            kT[hr, (ti + j) * 128:(ti + j + 1) * 128],
                                            qT[hr, so:so + 512],
                                            start=True, stop=True)
                                    pe = pla.tile([128, 1024], F32R, tag=f"pex{ti}", bufs=1)
                                    nc.scalar.activation(pe[:], pp[:], AF.Exp,
                                                         scale=0.125)
                                    pex.append(pe)
                                # ctx^T (+ Z row) accumulated over t
                                pc = psa.tile([65, 512], F32, tag="p", bufs=2)
                                for ti in range(0, NSC, 2):
                                    for j in range(2):
                                        nc.tensor.matmul(
                                            pc[:],
                                            vst[ti + j][:, h * 65:h * 65 + 65],
                                            pex[ti // 2][:, j * 512:(j + 1) * 512],
                                            start=(ti + j == 0),
                                            stop=(ti + j == NSC - 1))
                                # invZ
                                iz = pla.tile([128, 512], F32R, tag="invz", bufs=2)
                                with nc.allow_low_precision(reason="f32r invZ"):
                                    nc.vector.reciprocal(iz[0:1, :], pc[64:65, :])
                                # broadcast invZ over 64 partitions via PE
                                pb = psa.tile([64, 512], F32, tag="p", bufs=2)
                                nc.tensor.matmul(pb[:], t_cones[0:1, 0:64],
                                                 iz[0:1, :], start=True, stop=True)
                                bcs = pla.tile([64, 512], F32, tag="bcs", bufs=2)
                                nc.scalar.copy(bcs[:], pb[:])
                                # normalized ctx^T
                                nc.vector.tensor_tensor(
                                    ctxT[pair][hr, so:so + 512], pc[0:64, :],
                                    bcs[:], ALU.mult)

                    # tail linear + residual + LN1 stats
                    st_sum = pla.tile([128, NSC], F32, tag="st_sum")
                    st_sq = pla.tile([128, NSC], F32, tag="st_sq")
                    rr = []
                    for sc in range(NSC):
                        xpt = pla.tile([128, 512], F32, tag="xpt", bufs=2)
                        nc.sync.dma_start(xpt[:], d_xp[it, sc])
                        pw = psa.tile([128, 512], F32, tag="p", bufs=2)
                        for c in range(4):
                            nc.tensor.matmul(
                                pw[:], ctxT[c][:, sc * 128:(sc + 1) * 128],
                                t_wo[c][:], start=(c == 0), stop=(c == 3))
                        r = pla.tile([128, 512], F32, tag=f"rr{sc}")
                        nc.vector.tensor_tensor(r[:], pw[:], xpt[:], ALU.add)
                        nc.vector.reduce_sum(st_sum[:, sc:sc + 1], r[:],
                                             axis=mybir.AxisListType.X)
                        sq = pla.tile([128, 512], F32, tag="sqs", bufs=2)
                        nc.scalar.activation(sq[:], r[:], AF.Square,
                                             accum_out=st_sq[:, sc:sc + 1])
                        rr.append(r)
                    # stats -> mean, inv_std
                    mean8 = pla.tile([128, NSC], F32, tag="mean8")
                    inv8 = pla.tile([128, NSC], F32, tag="inv8")
                    msq = pla.tile([128, NSC], F32, tag="msq")
                    nc.vector.tensor_scalar_mul(mean8[:], st_sum[:], 1.0 / D)
                    nc.vector.tensor_scalar_mul(inv8[:], st_sq[:], 1.0 / D)
                    nc.vector.tensor_tensor(msq[:], mean8[:], mean8[:], ALU.mult)
                    nc.vector.tensor_tensor(inv8[:], inv8[:], msq[:], ALU.subtract)
                    nc.scalar.activation(inv8[:], inv8[:], AF.Sqrt, bias=t_eps[:])
                    nc.vector.reciprocal(inv8[:], inv8[:])
                    # normalize, spill h, transpose into hT
                    for sc in range(NSC):
                        ht_ = pla.tile([128, 512], F32, tag="hst", bufs=2)
                        nc.vector.tensor_scalar(
                            ht_[:], rr[sc][:], mean8[:, sc:sc + 1],
                            inv8[:, sc:sc + 1], ALU.subtract, ALU.mult)
                        nc.vector.tensor_tensor(ht_[:], ht_[:], t_gb[G1][:], ALU.mult)
                        nc.vector.tensor_tensor(ht_[:], ht_[:], t_gb[B1][:], ALU.add)
                        nc.sync.dma_start(h_dram[it][sc][:], ht_[:])
                        for dc in range(NDC):
                            pt = psa.tile([128, 128], F32, tag="ptr", bufs=2)
                            nc.tensor.transpose(pt[:], ht_[:, dc * 128:(dc + 1) * 128],
                                                t_ident[:])
                            nc.scalar.copy(
                                hT[it][dc][:, 4 + sc * 128: 4 + (sc + 1) * 128],
                                pt[:])
                    for dc in range(NDC):
                        nc.sync.dma_start(hT[it][dc][:, 0:4],
                                          d_czero[:, 0:4].bitcast(F32R))
                        nc.sync.dma_start(hT[it][dc][:, S + 4:S + 8],
                                          d_czero[:, 4:8].bitcast(F32R))

            # ---------------- Phase B + C: conv FFN + LN2 --------------------
            with ExitStack() as pb_:
                plb = pb_.enter_context(tc.tile_pool(name="pb", bufs=1))
                psb = pb_.enter_context(tc.tile_pool(name="psb", bufs=1, space="PSUM"))

                o2 = [[plb.tile([128, 512], F32, tag=f"o2_{it}_{sc}", name=f"o2_{it}_{sc}")
                       for sc in range(NSC)] for it in range(NIT)]

                for cdc in range(NCD):
                    w2t = plb.tile([128, KS * 512], F32R, tag="w2t", bufs=2)
                    nc.sync.dma_start(w2t[:], d_w2[cdc].bitcast(F32R))
                    w1t = []
                    for dc in range(NDC):
                        t = plb.tile([128, KS * 128], F32R, tag=f"w1t{dc}", bufs=2)
                        nc.sync.dma_start(t[:], d_w1[cdc, dc].bitcast(F32R))
                        w1t.append(t)
                    c1cur = []
                    for it in range(NIT):
                        c1t = plb.tile([128, S + 8], F32R, tag=f"c1t{it}", bufs=2)
                        nc.sync.dma_start(c1t[:, 0:4], d_czero[:, 0:4].bitcast(F32R))
                        nc.sync.dma_start(c1t[:, S + 4:S + 8],
                                          d_czero[:, 4:8].bitcast(F32R))
                        for scol in range(NCOL):
                            pc1 = psb.tile([128, 512], F32, tag="c1p", bufs=3)
                            idx = 0
                            for k in range(KS):
                                for dc in range(NDC):
                                    nc.tensor.matmul(
                                        pc1[:],
                                        w1t[dc][:, k * 128:(k + 1) * 128],
                                        hT[it][dc][:, scol * 512 + k:
                                                   scol * 512 + k + 512],
                                        start=(idx == 0), stop=(idx == 35))
                                    idx += 1
                            nc.scalar.activation(
                                c1t[:, 4 + scol * 512: 4 + (scol + 1) * 512],
                                pc1[:], AF.Relu,
                                bias=t_bc1s[:, cdc:cdc + 1])
                        c1cur.append(c1t)
                    for it in range(NIT):
                        for sc in range(NSC):
                            pc2 = psb.tile([128, 512], F32, tag="c2p", bufs=4)
                            for k in range(KS):
                                nc.tensor.matmul(
                                    pc2[:],
                                    c1cur[it][:, sc * 128 + k: sc * 128 + k + 128],
                                    w2t[:, k * 512:(k + 1) * 512],
                                    start=(k == 0), stop=(k == KS - 1))
                            if cdc == 0:
                                nc.vector.tensor_copy(o2[it][sc][:], pc2[:])
                            else:
                                nc.vector.tensor_tensor(o2[it][sc][:], pc2[:],
                                                        o2[it][sc][:], ALU.add)

                # Phase C: bias + relu + residual + LN2 + store
                for it in range(NIT):
                    st_sum = plb.tile([128, NSC], F32, tag="st2_sum")
                    st_sq = plb.tile([128, NSC], F32, tag="st2_sq")
                    rr = []
                    for sc in range(NSC):
                        t1 = plb.tile([128, 512], F32, tag="scr1", bufs=2)
                        nc.vector.tensor_tensor(t1[:], o2[it][sc][:], t_gb[BC2][:],
                                                ALU.add)
                        c2r = plb.tile([128, 512], F32, tag="scr2", bufs=2)
                        nc.scalar.activation(c2r[:], t1[:], AF.Relu)
                        hrl = plb.tile([128, 512], F32, tag="hrl", bufs=2)
                        nc.sync.dma_start(hrl[:], h_dram[it][sc][:])
                        r = plb.tile([128, 512], F32, tag=f"rr2{sc}")
                        nc.vector.tensor_tensor(r[:], c2r[:], hrl[:], ALU.add)
                        nc.vector.reduce_sum(st_sum[:, sc:sc + 1], r[:],
                                             axis=mybir.AxisListType.X)
                        sq = plb.tile([128, 512], F32, tag="sq2", bufs=2)
                        nc.scalar.activation(sq[:], r[:], AF.Square,
                                             accum_out=st_sq[:, sc:sc + 1])
                        rr.append(r)
                    mean8 = plb.tile([128, NSC], F32, tag="mean8b")
                    inv8 = plb.tile([128, NSC], F32, tag="inv8b")
                    msq = plb.tile([128, NSC], F32, tag="msqb")
                    nc.vector.tensor_scalar_mul(mean8[:], st_sum[:], 1.0 / D)
                    nc.vector.tensor_scalar_mul(inv8[:], st_sq[:], 1.0 / D)
                    nc.vector.tensor_tensor(msq[:], mean8[:], mean8[:], ALU.mult)
                    nc.vector.tensor_tensor(inv8[:], inv8[:], msq[:], ALU.subtract)
                    nc.scalar.activation(inv8[:], inv8[:], AF.Sqrt, bias=t_eps[:])
                    nc.vector.reciprocal(inv8[:], inv8[:])
                    for sc in range(NSC):
                        yt = plb.tile([128, 512], F32, tag="yt", bufs=2)
                        nc.vector.tensor_scalar(
                            yt[:], rr[sc][:], mean8[:, sc:sc + 1],
                            inv8[:, sc:sc + 1], ALU.subtract, ALU.mult)
                        nc.vector.tensor_tensor(yt[:], yt[:], t_gb[G2][:], ALU.mult)
                        nc.vector.tensor_tensor(yt[:], yt[:], t_gb[B2][:], ALU.add)
                        nc.sync.dma_start(d_y[it, sc], yt[:])

    nc.compile()
    _BUILT = nc
    return nc


def _prep_host(inputs):
    x = np.asarray(inputs["x"], np.float32)
    Wq = np.asarray(inputs["Wq"], np.float32)
    bq = np.asarray(inputs["bq"], np.float32)
    Wk = np.asarray(inputs["Wk"], np.float32)
    bk = np.asarray(inputs["bk"], np.float32)
    Wv = np.asarray(inputs["Wv"], np.float32)
    bv = np.asarray(inputs["bv"], np.float32)
    Wo = np.asarray(inputs["Wo"], np.float32)
    bo = np.asarray(inputs["bo"], np.float32)
    g1 = np.asarray(inputs["g1"], np.float32)
    b1 = np.asarray(inputs["b1"], np.float32)
    g2 = np.asarray(inputs["g2"], np.float32)
    b2 = np.asarray(inputs["b2"], np.float32)
    Wc1 = np.asarray(inputs["Wc1"], np.float32)
    bc1 = np.asarray(inputs["bc1"], np.float32)
    Wc2 = np.asarray(inputs["Wc2"], np.float32)
    bc2 = np.asarray(inputs["bc2"], np.float32)

    xT = np.ascontiguousarray(
        x.transpose(0, 2, 1).reshape(B, NDC, 128, S))
    xp = np.ascontiguousarray((x + bo[None, None, :]).reshape(B, NSC, 128, D))

    # Q/K pair weights: [proj, pair, d(128 within chunk), dc*128 + k2]
    wqk = np.zeros((2, 4, 128, 512), np.float32)
    for proj, W in ((0, Wq), (1, Wk)):
        for pair in range(4):
            blk = np.concatenate([W[2 * pair], W[2 * pair + 1]], axis=1)  # [D,128]
            wqk[proj, pair] = blk.reshape(NDC, 128, 128).transpose(1, 0, 2) \
                                 .reshape(128, 512)
    bqk = np.zeros((128, 8), np.float32)
    for proj, b in ((0, bq), (1, bk)):
        for pair in range(4):
            bqk[:, proj * 4 + pair] = np.concatenate(
                [b[2 * pair], b[2 * pair + 1]])

    # V weights with per-head ones column: [dc, d(128), h*65 + j]
    wv = np.zeros((NDC, 128, 520), np.float32)
    bvrow = np.zeros((128, 520), np.float32)
    for h in range(H):
        wv[:, :, h * 65:h * 65 + 64] = Wv[h].reshape(NDC, 128, 64)
        bvrow[0, h * 65:h * 65 + 64] = bv[h]
        bvrow[0, h * 65 + 64] = 1.0

    wo = np.ascontiguousarray(Wo.reshape(4, 128, 512))

    # conv1: w1[cdc, dc, d(128), k*128 + cd(128)] = Wc1[cdc*128+cd, dc*128+d, k]
    w1 = np.ascontiguousarray(
        Wc1.reshape(NCD, 128, NDC, 128, KS).transpose(0, 2, 3, 4, 1)
           .reshape(NCD, NDC, 128, KS * 128))
    # conv2: w2[cdc, cd(128), k*512 + dout] = Wc2[dout, cdc*128+cd, k]
    w2 = np.ascontiguousarray(
        Wc2.reshape(D, NCD, 128, KS).transpose(1, 2, 3, 0)
           .reshape(NCD, 128, KS * 512))
    bc1s = np.ascontiguousarray(bc1.reshape(NCD, 128).T)

    gb = np.stack([np.tile(v[None, :], (128, 1))
                   for v in (g1, b1, g2, b2, bc2)]).astype(np.float32)
    cones = np.ones((128, 128), np.float32)
    czero = np.zeros((128, 8), np.float32)

    shared = dict(wqk=wqk, bqk=bqk, wv=wv, bvrow=bvrow, wo=wo, w1=w1, w2=w2,
                  bc1s=bc1s, gb=gb, cones=cones, czero=czero)
    in_maps = []
    for c in range(NCORES):
        m = dict(shared)
        m["xT"] = np.ascontiguousarray(xT[c * NIT:(c + 1) * NIT])
        m["xp"] = np.ascontiguousarray(xp[c * NIT:(c + 1) * NIT])
        in_maps.append(m)
    return in_maps


def run(inputs, trace=False, **trace_kwargs):
    nc = _build()
    from concourse.bass_utils import run_bass_kernel_spmd
    in_maps = _prep_host(inputs)
    res = run_bass_kernel_spmd(nc, in_maps, core_ids=list(range(NCORES)),
                               trace=trace, **trace_kwargs)
    y = np.concatenate([res.results[c]["y"].reshape(NIT, S, D)
                        for c in range(NCORES)], axis=0)
    return y, res


def kernel(**inputs):
    y, _ = run(inputs, trace=False)
    return y


# revision 10
# speedup vs baseline: 1.0659x; 1.0659x over previous
"""Trainium2 Bass kernel for the FFT-block (attention + conv FFN) problem.

Sharding: data-parallel over batch. B=16 items across 8 cores -> 2 items/core.
Each core runs the full block for its items; no collectives.

Per item:
  - attention via scores^T = K Q^T (softmax sums land on the partition axis and
    are folded into the ctx matmul through a ones-column appended to V); the
    per-head 1/Z normalization is broadcast across partitions with a K=1 PE
    matmul.  Attention matmuls run in fp32r (tf32-like, fp32 accumulate);
    softmax weights and V are bf16.
  - convs are 9 shifted matmuls over transposed activations hT [D, S_pad] in
    bf16 (weights+activations), fp32 PSUM accumulation and fp32 o2 accumulator.
  - emission order software-pipelines item1's attention into item0's conv
    stream so the PE never drains (HAM stays at K=8/8).
"""
import sys, types
import numpy as np

B, S, D = 16, 1024, 512
H, DK = 8, 64
CD, KS = 2048, 9
EPS = 1e-5
NCORES = 8
NIT = B // NCORES
NDC = D // 128             # 4 d-chunks
NSC = S // 128             # 8 s-chunks
NCOL = S // 512            # 2 s-cols
NCD = CD // 128            # 16 cd-chunks


def _install_ntff_hook():
    try:
        from antenv.axon_hooks import get_axon_ntff_profile_hook  # noqa
        return
    except ImportError:
        pass
    try:
        from trn_agent_boot.trn_boot import _ntff_profile_via_ctypes
        mod = types.ModuleType('antenv.axon_hooks')
        hook = _ntff_profile_via_ctypes('/opt/axon/libaxon_pjrt.so')
        mod.get_axon_ntff_profile_hook = lambda: hook
        sys.modules['antenv.axon_hooks'] = mod
    except Exception:
        pass


_BUILT = None


def _build():
    global _BUILT
    if _BUILT is not None:
        return _BUILT
    _install_ntff_hook()
    import concourse.bacc as bacc
    import concourse.mybir as mybir
    from concourse import tile
    from concourse.masks import make_identity
    from contextlib import ExitStack

    F32 = mybir.dt.float32
    F32R = mybir.dt.float32r
    BF16 = mybir.dt.bfloat16
    AF = mybir.ActivationFunctionType
    ALU = mybir.AluOpType
    AX = mybir.AxisListType

    nc = bacc.Bacc("TRN2", target_bir_lowering=False, debug=False,
                   num_devices=NCORES)

    # ---- DRAM I/O (per core) ----
    d_xT = nc.dram_tensor("xT", [NIT, NDC, 128, S], F32, kind="ExternalInput")
    d_xp = nc.dram_tensor("xp", [NIT, NSC, 128, D], F32, kind="ExternalInput")
    d_wqk = nc.dram_tensor("wqk", [2, 4, 128, 512], F32, kind="ExternalInput")
    d_bqk = nc.dram_tensor("bqk", [128, 8], F32, kind="ExternalInput")
    d_wv = nc.dram_tensor("wv", [NDC, 128, 520], F32, kind="ExternalInput")
    d_bvrow = nc.dram_tensor("bvrow", [128, 520], F32, kind="ExternalInput")
    d_wo = nc.dram_tensor("wo", [4, 128, 512], F32, kind="ExternalInput")
    d_w1 = nc.dram_tensor("w1", [NCD, NDC, 128, KS * 128], BF16,
                          kind="ExternalInput")
    d_w2 = nc.dram_tensor("w2", [NCD, 128, KS * 512], BF16,
                          kind="ExternalInput")
    d_bc1s = nc.dram_tensor("bc1s", [128, NCD], F32, kind="ExternalInput")
    d_gb = nc.dram_tensor("gb", [5, 128, 512], F32, kind="ExternalInput")
    d_cones = nc.dram_tensor("cones", [128, 128], F32, kind="ExternalInput")
    d_czero = nc.dram_tensor("czero", [128, 8], BF16, kind="ExternalInput")
    d_y = nc.dram_tensor("y", [NIT, NSC, 128, D], F32, kind="ExternalOutput")

    G1, B1, G2, B2, BC2 = range(5)

    with tile.TileContext(nc) as tc:
        est = ExitStack()
        with est:
            cp = est.enter_context(tc.tile_pool(name="const", bufs=1))
            pl = est.enter_context(tc.tile_pool(name="work", bufs=1))
            ps = est.enter_context(tc.tile_pool(name="psum", bufs=1, space="PSUM"))
            dp = est.enter_context(tc.tile_pool(name="dramp", bufs=1, space="DRAM"))

            h_dram = [[dp.tile([128, D], F32, tag=f"hd{it}_{sc}",
                               name=f"hd{it}_{sc}")
                       for sc in range(NSC)] for it in range(NIT)]

            # ---- constants ----
            t_bqk = cp.tile([128, 8], F32, tag="bqk")
            nc.sync.dma_start(t_bqk[:], d_bqk[:])
            t_gb = []
            for i in range(5):
                t = cp.tile([128, 512], F32, tag=f"gb{i}", name=f"gb{i}")
                nc.sync.dma_start(t[:], d_gb[i])
                t_gb.append(t)
            t_bc1s = cp.tile([128, NCD], F32, tag="bc1s")
            nc.sync.dma_start(t_bc1s[:], d_bc1s[:])
            t_ident = cp.tile([128, 128], F32, tag="ident")
            make_identity(nc, t_ident[:])
            t_cones = cp.tile([128, 128], F32R, tag="cones")
            nc.sync.dma_start(t_cones[:], d_cones[:].bitcast(F32R))
            t_czero = cp.tile([128, 8], BF16, tag="czero")
            nc.sync.dma_start(t_czero[:], d_czero[:])
            t_eps = cp.tile([128, 1], F32, tag="eps")
            nc.vector.memset(t_eps[:], EPS)
            t_wv = []
            for dc in range(NDC):
                t = cp.tile([128, 520], F32R, tag=f"wv{dc}", name=f"wv{dc}")
                nc.sync.dma_start(t[:], d_wv[dc].bitcast(F32R))
                t_wv.append(t)
            t_bvrow = cp.tile([128, 520], F32R, tag="bvrow")
            nc.sync.dma_start(t_bvrow[:], d_bvrow[:].bitcast(F32R))
            t_wo = []
            for c in range(4):
                t = cp.tile([128, 512], F32R, tag=f"wo{c}", name=f"wo{c}")
                nc.sync.dma_start(t[:], d_wo[c].bitcast(F32R))
                t_wo.append(t)

            # persistent hT tiles (bf16, padded s)
            hT = [[pl.tile([128, S + 8], BF16, tag=f"ht{it}_{dc}",
                           name=f"ht{it}_{dc}")
                   for dc in range(NDC)] for it in range(NIT)]

            state = [dict() for _ in range(NIT)]

            # ================= emit helpers =================
            def emit_proj(it):
                """xT load + V/Q/K projections for one item (dense PE block)."""
                st = state[it]
                xt = []
                for dc in range(NDC):
                    t = pl.tile([128, S], F32R, tag=f"xt{dc}", name=f"xt{dc}")
                    nc.sync.dma_start(t[:], d_xT[it, dc].bitcast(F32R))
                    xt.append(t)
                st["xt"] = xt
                vst = []
                for tc_i in range(NSC):
                    vt = pl.tile([128, 520], BF16, tag=f"vst{tc_i}",
                                 name=f"vst{tc_i}")
                    for half in range(2):
                        colo = half * 260
                        pv = ps.tile([128, 260], F32, tag="pp", bufs=3)
                        for dc in range(NDC):
                            nc.tensor.matmul(
                                pv[:], xt[dc][:, tc_i * 128:(tc_i + 1) * 128],
                                t_wv[dc][:, colo:colo + 260],
                                start=(dc == 0), stop=False)
                        nc.tensor.matmul(
                            pv[:], t_cones[0:1, 0:128],
                            t_bvrow[0:1, colo:colo + 260],
                            start=False, stop=True)
                        nc.vector.tensor_copy(vt[:, colo:colo + 260], pv[:])
                    vst.append(vt)
                st["vst"] = vst
                qkt = {}
                for pair in range(4):
                    for proj in range(2):
                        wt = pl.tile([128, 512], F32R, tag=f"wqk{proj}",
                                     bufs=2, name="wt")
                        nc.sync.dma_start(wt[:], d_wqk[proj, pair].bitcast(F32R))
                        qt = pl.tile([128, S], BF16, tag=f"qk{proj}{pair}",
                                     name="qt")
                        for scol in range(NCOL):
                            pq = ps.tile([128, 512], F32, tag="pp", bufs=3)
                            for dc in range(NDC):
                                nc.tensor.matmul(
                                    pq[:], wt[:, dc * 128:(dc + 1) * 128],
                                    xt[dc][:, scol * 512:(scol + 1) * 512],
                                    start=(dc == 0), stop=(dc == NDC - 1))
                            nc.vector.tensor_scalar_add(
                                qt[:, scol * 512:(scol + 1) * 512], pq[:],
                                t_bqk[:, proj * 4 + pair:proj * 4 + pair + 1])
                        qkt[(proj, pair)] = qt
                st["qkt"] = qkt

            def emit_heads_pair(it, pair):
                st = state[it]
                if pair == 0:
                    st["ctxT"] = [pl.tile([128, S], F32R, tag=f"ct{c}",
                                          name=f"ct{c}") for c in range(4)]
                qT = st["qkt"][(0, pair)]
                kT = st["qkt"][(1, pair)]
                vst = st["vst"]
                ctxT = st["ctxT"]
                for sub in range(2):
                    h = 2 * pair + sub
                    hr = slice(sub * 64, sub * 64 + 64)
                    for scol in range(NCOL):
                        so = scol * 512
                        pex = []
                        for ti in range(NSC):
                            pp = ps.tile([128, 512], F32, tag="pp", bufs=3)
                            nc.tensor.matmul(
                                pp[:], kT[hr, ti * 128:(ti + 1) * 128],
                                qT[hr, so:so + 512], start=True, stop=True)
                            pe = pl.tile([128, 512], BF16, tag=f"pex{ti}",
                                         bufs=1, name="pe")
                            nc.scalar.activation(pe[:], pp[:], AF.Exp,
                                                 scale=0.125)
                            pex.append(pe)
                        pc = ps.tile([65, 512], F32, tag="pc", bufs=1)
                        for ti in range(NSC):
                            nc.tensor.matmul(
                                pc[:], vst[ti][:, h * 65:h * 65 + 65],
                                pex[ti][:], start=(ti == 0),
                                stop=(ti == NSC - 1))
                        iz = pl.tile([128, 512], F32R, tag="bcs", bufs=2,
                                     name="iz")
                        with nc.allow_low_precision(reason="f32r invZ"):
                            nc.vector.reciprocal(iz[0:1, :], pc[64:65, :])
                        pb = ps.tile([64, 512], F32, tag="pp", bufs=3)
                        nc.tensor.matmul(pb[:], t_cones[0:1, 0:64], iz[0:1, :],
                                         start=True, stop=True)
                        bcs = pl.tile([64, 512], F32, tag="bcs", bufs=2,
                                      name="bcs")
                        nc.vector.tensor_copy(bcs[:], pb[:])
                        nc.vector.tensor_tensor(
                            ctxT[pair][hr, so:so + 512], pc[0:64, :],
                            bcs[:], ALU.mult)

            def emit_tail(it):
                """Wo + residual + LN1 + transpose into hT (+ h spill)."""
                st = state[it]
                ctxT = st["ctxT"]
                st_sum = pl.tile([128, NSC], F32, tag="st_sum", bufs=2)
                st_sq = pl.tile([128, NSC], F32, tag="st_sq", bufs=2)
                rr = []
                for sc in range(NSC):
                    xpt = pl.tile([128, 512], F32, tag="xpt", bufs=2)
                    nc.sync.dma_start(xpt[:], d_xp[it, sc])
                    pw = ps.tile([128, 512], F32, tag="pc", bufs=1)
                    for c in range(4):
                        nc.tensor.matmul(
                            pw[:], ctxT[c][:, sc * 128:(sc + 1) * 128],
                            t_wo[c][:], start=(c == 0), stop=(c == 3))
                    r = pl.tile([128, 512], F32, tag=f"res{sc}", name="r")
                    nc.vector.tensor_tensor(r[:], pw[:], xpt[:], ALU.add)
                    nc.vector.reduce_sum(st_sum[:, sc:sc + 1], r[:], axis=AX.X)
                    sq = pl.tile([128, 512], BF16, tag="sqs", bufs=2, name="sq")
                    nc.scalar.activation(sq[:], r[:], AF.Square,
                                         accum_out=st_sq[:, sc:sc + 1])
                    rr.append(r)
                mean8 = pl.tile([128, NSC], F32, tag="mean8", bufs=2)
                inv8 = pl.tile([128, NSC], F32, tag="inv8", bufs=2)
                msq = pl.tile([128, NSC], F32, tag="msq", bufs=2)
                nc.vector.tensor_scalar_mul(mean8[:], st_sum[:], 1.0 / D)
                nc.vector.tensor_scalar_mul(inv8[:], st_sq[:], 1.0 / D)
                nc.vector.tensor_tensor(msq[:], mean8[:], mean8[:], ALU.mult)
                nc.vector.tensor_tensor(inv8[:], inv8[:], msq[:], ALU.subtract)
                nc.scalar.activation(inv8[:], inv8[:], AF.Sqrt, bias=t_eps[:])
                nc.vector.reciprocal(inv8[:], inv8[:])
                for sc in range(NSC):
                    ht_ = pl.tile([128, 512], F32, tag="hst", bufs=2, name="h_")
                    nc.vector.tensor_scalar(
                        ht_[:], rr[sc][:], mean8[:, sc:sc + 1],
                        inv8[:, sc:sc + 1], ALU.subtract, ALU.mult)
                    nc.vector.tensor_tensor(ht_[:], ht_[:], t_gb[G1][:], ALU.mult)
                    nc.vector.tensor_tensor(ht_[:], ht_[:], t_gb[B1][:], ALU.add)
                    nc.sync.dma_start(h_dram[it][sc][:], ht_[:])
                    for dc in range(NDC):
                        pt = ps.tile([128, 128], F32, tag="pp", bufs=3)
                        nc.tensor.transpose(pt[:], ht_[:, dc * 128:(dc + 1) * 128],
                                            t_ident[:])
                        nc.scalar.copy(
                            hT[it][dc][:, 4 + sc * 128: 4 + (sc + 1) * 128],
                            pt[:])
                for dc in range(NDC):
                    nc.sync.dma_start(hT[it][dc][:, 0:4], d_czero[:, 0:4])
                    nc.sync.dma_start(hT[it][dc][:, S + 4:S + 8],
                                      d_czero[:, 4:8])

            o2 = [[None] * NSC for _ in range(NIT)]

            def emit_conv_chunk(it, cdc):
                w2t = pl.tile([128, KS * 512], BF16, tag="w2t", bufs=2,
                              name="w2t")
                nc.sync.dma_start(w2t[:], d_w2[cdc])
                w1t = []
                for dc in range(NDC):
                    t = pl.tile([128, KS * 128], BF16, tag=f"w1t{dc}", bufs=2,
                                name="w1t")
                    nc.sync.dma_start(t[:], d_w1[cdc, dc])
                    w1t.append(t)
                c1t = pl.tile([128, S + 8], BF16, tag="c1t", bufs=2, name="c1t")
                nc.sync.dma_start(c1t[:, 0:4], d_czero[:, 0:4])
                nc.sync.dma_start(c1t[:, S + 4:S + 8], d_czero[:, 4:8])
                for scol in range(NCOL):
                    pc1 = ps.tile([128, 512], F32, tag="c1p", bufs=2)
                    idx = 0
                    for k in range(KS):
                        for dc in range(NDC):
                            nc.tensor.matmul(
                                pc1[:], w1t[dc][:, k * 128:(k + 1) * 128],
                                hT[it][dc][:, scol * 512 + k:
                                           scol * 512 + k + 512],
                                start=(idx == 0), stop=(idx == 35))
                            idx += 1
                    nc.scalar.activation(
                        c1t[:, 4 + scol * 512: 4 + (scol + 1) * 512],
                        pc1[:], AF.Relu, bias=t_bc1s[:, cdc:cdc + 1])
                for sc in range(NSC):
                    pc2 = ps.tile([128, 512], F32, tag="c2p", bufs=2)
                    for k in range(KS):
                        nc.tensor.matmul(
                            pc2[:], c1t[:, sc * 128 + k: sc * 128 + k + 128],
                            w2t[:, k * 512:(k + 1) * 512],
                            start=(k == 0), stop=(k == KS - 1))
                    if cdc == 0:
                        t = pl.tile([128, 512], F32, tag=f"o2_{sc}",
                                    name=f"o2_{sc}")
                        o2[it][sc] = t
                        nc.vector.tensor_copy(t[:], pc2[:])
                    else:
                        nc.vector.tensor_tensor(o2[it][sc][:], pc2[:],
                                                o2[it][sc][:], ALU.add)

            def emit_ln2(it):
                st_sum = pl.tile([128, NSC], F32, tag="st_sum", bufs=2)
                st_sq = pl.tile([128, NSC], F32, tag="st_sq", bufs=2)
                rr = []
                for sc in range(NSC):
                    t1 = pl.tile([128, 512], F32, tag="hst", bufs=2)
                    nc.vector.tensor_tensor(t1[:], o2[it][sc][:], t_gb[BC2][:],
                                            ALU.add)
                    nc.scalar.activation(t1[:], t1[:], AF.Relu)
                    hrl = pl.tile([128, 512], F32, tag="xpt", bufs=2)
                    nc.sync.dma_start(hrl[:], h_dram[it][sc][:])
                    r = pl.tile([128, 512], F32, tag=f"res{sc}", name="r2")
                    nc.vector.tensor_tensor(r[:], t1[:], hrl[:], ALU.add)
                    nc.vector.reduce_sum(st_sum[:, sc:sc + 1], r[:], axis=AX.X)
                    sq = pl.tile([128, 512], BF16, tag="sqs", bufs=2, name="sq2")
                    nc.scalar.activation(sq[:], r[:], AF.Square,
                                         accum_out=st_sq[:, sc:sc + 1])
                    rr.append(r)
                mean8 = pl.tile([128, NSC], F32, tag="mean8", bufs=2)
                inv8 = pl.tile([128, NSC], F32, tag="inv8", bufs=2)
                msq = pl.tile([128, NSC], F32, tag="msq", bufs=2)
                nc.vector.tensor_scalar_mul(mean8[:], st_sum[:], 1.0 / D)
                nc.vector.tensor_scalar_mul(inv8[:], st_sq[:], 1.0 / D)
                nc.vector.tensor_tensor(msq[:], mean8[:], mean8[:], ALU.mult)
                nc.vector.tensor_tensor(inv8[:], inv8[:], msq[:], ALU.subtract)
                nc.scalar.activation(inv8[:], inv8[:], AF.Sqrt, bias=t_eps[:])
                nc.vector.reciprocal(inv8[:], inv8[:])
                for sc in range(NSC):
                    yt = pl.tile([128, 512], F32, tag="hst", bufs=2)
                    nc.vector.tensor_scalar(
                        yt[:], rr[sc][:], mean8[:, sc:sc + 1],
                        inv8[:, sc:sc + 1], ALU.subtract, ALU.mult)
                    nc.vector.tensor_tensor(yt[:], yt[:], t_gb[G2][:], ALU.mult)
                    nc.vector.tensor_tensor(yt[:], yt[:], t_gb[B2][:], ALU.add)
                    nc.sync.dma_start(d_y[it, sc], yt[:])

            # ================= emission order =================
            emit_proj(0)
            for pair in range(4):
                emit_heads_pair(0, pair)
            emit_tail(0)
            emit_proj(1)
            for cdc in range(NCD):
                emit_conv_chunk(0, cdc)
                if cdc < 4:
                    emit_heads_pair(1, cdc)
                elif cdc == 7:
                    emit_tail(1)
            emit_ln2(0)
            for cdc in range(NCD):
                emit_conv_chunk(1, cdc)
            emit_ln2(1)

    nc.compile()
    _BUILT = nc
    return nc


def _prep_host(inputs):
    import ml_dtypes
    bf16 = ml_dtypes.bfloat16
    x = np.asarray(inputs["x"], np.float32)
    Wq = np.asarray(inputs["Wq"], np.float32)
    bq = np.asarray(inputs["bq"], np.float32)
    Wk = np.asarray(inputs["Wk"], np.float32)
    bk = np.asarray(inputs["bk"], np.float32)
    Wv = np.asarray(inputs["Wv"], np.float32)
    bv = np.asarray(inputs["bv"], np.float32)
    Wo = np.asarray(inputs["Wo"], np.float32)
    bo = np.asarray(inputs["bo"], np.float32)
    g1 = np.asarray(inputs["g1"], np.float32)
    b1 = np.asarray(inputs["b1"], np.float32)
    g2 = np.asarray(inputs["g2"], np.float32)
    b2 = np.asarray(inputs["b2"], np.float32)
    Wc1 = np.asarray(inputs["Wc1"], np.float32)
    bc1 = np.asarray(inputs["bc1"], np.float32)
    Wc2 = np.asarray(inputs["Wc2"], np.float32)
    bc2 = np.asarray(inputs["bc2"], np.float32)

    xT = np.ascontiguousarray(x.transpose(0, 2, 1).reshape(B, NDC, 128, S))
    xp = np.ascontiguousarray((x + bo[None, None, :]).reshape(B, NSC, 128, D))

    wqk = np.zeros((2, 4, 128, 512), np.float32)
    for proj, W in ((0, Wq), (1, Wk)):
        for pair in range(4):
            blk = np.concatenate([W[2 * pair], W[2 * pair + 1]], axis=1)
            wqk[proj, pair] = blk.reshape(NDC, 128, 128).transpose(1, 0, 2) \
                                 .reshape(128, 512)
    bqk = np.zeros((128, 8), np.float32)
    for proj, b in ((0, bq), (1, bk)):
        for pair in range(4):
            bqk[:, proj * 4 + pair] = np.concatenate(
                [b[2 * pair], b[2 * pair + 1]])

    wv = np.zeros((NDC, 128, 520), np.float32)
    bvrow = np.zeros((128, 520), np.float32)
    for h in range(H):
        wv[:, :, h * 65:h * 65 + 64] = Wv[h].reshape(NDC, 128, 64)
        bvrow[0, h * 65:h * 65 + 64] = bv[h]
        bvrow[0, h * 65 + 64] = 1.0

    wo = np.ascontiguousarray(Wo.reshape(4, 128, 512))

    w1 = np.ascontiguousarray(
        Wc1.reshape(NCD, 128, NDC, 128, KS).transpose(0, 2, 3, 4, 1)
           .reshape(NCD, NDC, 128, KS * 128)).astype(bf16)
    w2 = np.ascontiguousarray(
        Wc2.reshape(D, NCD, 128, KS).transpose(1, 2, 3, 0)
           .reshape(NCD, 128, KS * 512)).astype(bf16)
    bc1s = np.ascontiguousarray(bc1.reshape(NCD, 128).T)

    gb = np.stack([np.tile(v[None, :], (128, 1))
                   for v in (g1, b1, g2, b2, bc2)]).astype(np.float32)
    cones = np.ones((128, 128), np.float32)
    czero = np.zeros((128, 8), bf16)

    shared = dict(wqk=wqk, bqk=bqk, wv=wv, bvrow=bvrow, wo=wo,
                  w1=w1, w2=w2, bc1s=bc1s, gb=gb, cones=cones, czero=czero)
    in_maps = []
    for c in range(NCORES):
        m = dict(shared)
        m["xT"] = np.ascontiguousarray(xT[c * NIT:(c + 1) * NIT])
        m["xp"] = np.ascontiguousarray(xp[c * NIT:(c + 1) * NIT])
        in_maps.append(m)
    return in_maps


def run(inputs, trace=False, **trace_kwargs):
    nc = _build()
    from concourse.bass_utils import run_bass_kernel_spmd
    in_maps = _prep_host(inputs)
    res = run_bass_kernel_spmd(nc, in_maps, core_ids=list(range(NCORES)),
                               trace=trace, **trace_kwargs)
    y = np.concatenate([res.results[c]["y"].reshape(NIT, S, D)
                        for c in range(NCORES)], axis=0)
    return y, res


def kernel(**inputs):
    y, _ = run(inputs, trace=False)
    return y


# revision 17
# speedup vs baseline: 1.1662x; 1.0941x over previous
"""Trainium2 Bass kernel for the FFT-block (attention + conv FFN) problem.

Sharding: data-parallel over batch. B=16 items across 8 cores -> 2 items/core.
Each core runs the full block for its items; no collectives.

Per item:
  - attention via scores^T = K Q^T (softmax sums land on the partition axis and
    are folded into the ctx matmul through a ones-column appended to V); the
    per-head 1/Z normalization is broadcast across partitions with a K=1 PE
    matmul.  Attention matmuls run in fp32r (tf32-like, fp32 accumulate);
    softmax weights and V are bf16.
  - convs are 9 shifted matmuls over transposed activations hT [D, S_pad] in
    bf16 (weights+activations), fp32 PSUM accumulation and fp32 o2 accumulator.
  - emission order software-pipelines item1's attention into item0's conv
    stream so the PE never drains (HAM stays at K=8/8).
"""
import sys, types
import numpy as np

B, S, D = 16, 1024, 512
H, DK = 8, 64
CD, KS = 2048, 9
EPS = 1e-5
NCORES = 8
NIT = B // NCORES
NDC = D // 128             # 4 d-chunks
NSC = S // 128             # 8 s-chunks
NCOL = S // 512            # 2 s-cols
NCD = CD // 128            # 16 cd-chunks


def _install_ntff_hook():
    try:
        from antenv.axon_hooks import get_axon_ntff_profile_hook  # noqa
        return
    except ImportError:
        pass
    try:
        from trn_agent_boot.trn_boot import _ntff_profile_via_ctypes
        mod = types.ModuleType('antenv.axon_hooks')
        hook = _ntff_profile_via_ctypes('/opt/axon/libaxon_pjrt.so')
        mod.get_axon_ntff_profile_hook = lambda: hook
        sys.modules['antenv.axon_hooks'] = mod
    except Exception:
        pass


_BUILT = None


def _build():
    global _BUILT
    if _BUILT is not None:
        return _BUILT
    _install_ntff_hook()
    import concourse.bacc as bacc
    import concourse.mybir as mybir
    from concourse import tile
    from concourse.masks import make_identity
    from contextlib import ExitStack

    F32 = mybir.dt.float32
    F32R = mybir.dt.float32r
    BF16 = mybir.dt.bfloat16
    AF = mybir.ActivationFunctionType
    ALU = mybir.AluOpType
    AX = mybir.AxisListType

    nc = bacc.Bacc("TRN2", target_bir_lowering=False, debug=False,
                   num_devices=NCORES)

    # ---- DRAM I/O (per core) ----
    d_xT = nc.dram_tensor("xT", [NIT, NDC, 128, S], F32, kind="ExternalInput")
    d_xp = nc.dram_tensor("xp", [NIT, NSC, 128, D], F32, kind="ExternalInput")
    d_wqk = nc.dram_tensor("wqk", [2, 4, 128, 512], F32, kind="ExternalInput")
    d_bqk = nc.dram_tensor("bqk", [128, 8], F32, kind="ExternalInput")
    d_wv = nc.dram_tensor("wv", [NDC, 128, 520], F32, kind="ExternalInput")
    d_bvrow = nc.dram_tensor("bvrow", [128, 520], F32, kind="ExternalInput")
    d_wo = nc.dram_tensor("wo", [4, 128, 512], F32, kind="ExternalInput")
    d_w1 = nc.dram_tensor("w1", [NCD, NDC, 128, KS * 128], BF16,
                          kind="ExternalInput")
    d_w2 = nc.dram_tensor("w2", [NCD, 128, KS * 512], BF16,
                          kind="ExternalInput")
    d_bc1s = nc.dram_tensor("bc1s", [128, NCD], F32, kind="ExternalInput")
    d_gb = nc.dram_tensor("gb", [5, 128, 512], F32, kind="ExternalInput")
    d_cones = nc.dram_tensor("cones", [128, 128], F32, kind="ExternalInput")
    d_czero = nc.dram_tensor("czero", [128, 8], BF16, kind="ExternalInput")
    d_y = nc.dram_tensor("y", [NIT, NSC, 128, D], F32, kind="ExternalOutput")

    G1, B1, G2, B2, BC2 = range(5)

    with tile.TileContext(nc) as tc:
        est = ExitStack()
        with est:
            cp = est.enter_context(tc.tile_pool(name="const", bufs=1))
            pl = est.enter_context(tc.tile_pool(name="work", bufs=1))
            ps = est.enter_context(tc.tile_pool(name="psum", bufs=1, space="PSUM"))
            dp = est.enter_context(tc.tile_pool(name="dramp", bufs=1, space="DRAM"))

            h_dram = [[dp.tile([128, D], F32, tag=f"hd{it}_{sc}",
                               name=f"hd{it}_{sc}")
                       for sc in range(NSC)] for it in range(NIT)]

            # ---- constants ----
            t_bqk = cp.tile([128, 8], F32, tag="bqk")
            nc.sync.dma_start(t_bqk[:], d_bqk[:])
            t_gb = []
            for i in range(5):
                t = cp.tile([128, 512], F32, tag=f"gb{i}", name=f"gb{i}")
                nc.sync.dma_start(t[:], d_gb[i])
                t_gb.append(t)
            t_bc1s = cp.tile([128, NCD], F32, tag="bc1s")
            nc.sync.dma_start(t_bc1s[:], d_bc1s[:])
            t_ident = cp.tile([128, 128], F32, tag="ident")
            make_identity(nc, t_ident[:])
            t_cones = cp.tile([128, 128], F32R, tag="cones")
            nc.sync.dma_start(t_cones[:], d_cones[:].bitcast(F32R))
            t_czero = cp.tile([128, 8], BF16, tag="czero")
            nc.sync.dma_start(t_czero[:], d_czero[:])
            t_eps = cp.tile([128, 1], F32, tag="eps")
            nc.vector.memset(t_eps[:], EPS)
            t_wv = []
            for dc in range(NDC):
                t = cp.tile([128, 520], F32R, tag=f"wv{dc}", name=f"wv{dc}")
                nc.sync.dma_start(t[:], d_wv[dc].bitcast(F32R))
                t_wv.append(t)
            t_bvrow = cp.tile([128, 520], F32R, tag="bvrow")
            nc.sync.dma_start(t_bvrow[:], d_bvrow[:].bitcast(F32R))
            t_wo = []
            for c in range(4):
                t = cp.tile([128, 512], F32R, tag=f"wo{c}", name=f"wo{c}")
                nc.sync.dma_start(t[:], d_wo[c].bitcast(F32R))
                t_wo.append(t)

            # persistent hT tiles (bf16, padded s)
            hT = [[pl.tile([128, S + 8], BF16, tag=f"ht{it}_{dc}",
                           name=f"ht{it}_{dc}")
                   for dc in range(NDC)] for it in range(NIT)]

            state = [dict() for _ in range(NIT)]

            # ================= emit helpers =================
            def emit_x(it):
                st = state[it]
                xt = []
                for dc in range(NDC):
                    t = pl.tile([128, S], F32R, tag=f"xt{dc}", name=f"xt{dc}")
                    nc.sync.dma_start(t[:], d_xT[it, dc].bitcast(F32R))
                    xt.append(t)
                st["xt"] = xt
                st["qkt"] = {}

            def emit_v(it):
                """V projection for one item (dense PE block)."""
                st = state[it]
                xt = st["xt"]
                vst = []
                for tc_i in range(NSC):
                    vt = pl.tile([128, 520], BF16, tag=f"vst{tc_i}",
                                 name=f"vst{tc_i}")
                    for half in range(2):
                        colo = half * 260
                        pv = ps.tile([128, 260], F32, tag="pp", bufs=2)
                        for dc in range(NDC):
                            nc.tensor.matmul(
                                pv[:], xt[dc][:, tc_i * 128:(tc_i + 1) * 128],
                                t_wv[dc][:, colo:colo + 260],
                                start=(dc == 0), stop=False)
                        nc.tensor.matmul(
                            pv[:], t_cones[0:1, 0:128],
                            t_bvrow[0:1, colo:colo + 260],
                            start=False, stop=True)
                        nc.vector.tensor_copy(vt[:, colo:colo + 260], pv[:])
                    vst.append(vt)
                st["vst"] = vst

            def emit_qk(it, pair):
                st = state[it]
                xt = st["xt"]
                for proj in range(2):
                    wt = pl.tile([128, 512], F32R, tag=f"wqk{proj}",
                                 bufs=2, name="wt")
                    nc.sync.dma_start(wt[:], d_wqk[proj, pair].bitcast(F32R))
                    qt = pl.tile([128, S], BF16, tag=f"qk{proj}{pair}",
                                 name="qt")
                    for scol in range(NCOL):
                        pq = ps.tile([128, 512], F32, tag="pp", bufs=2)
                        for dc in range(NDC):
                            nc.tensor.matmul(
                                pq[:], wt[:, dc * 128:(dc + 1) * 128],
                                xt[dc][:, scol * 512:(scol + 1) * 512],
                                start=(dc == 0), stop=(dc == NDC - 1))
                        nc.vector.tensor_scalar_add(
                            qt[:, scol * 512:(scol + 1) * 512], pq[:],
                            t_bqk[:, proj * 4 + pair:proj * 4 + pair + 1])
                    st["qkt"][(proj, pair)] = qt

            def emit_heads_pair(it, pair):
                st = state[it]
                if pair == 0:
                    st["ctxT"] = [pl.tile([128, S], F32R, tag=f"ct{c}",
                                          name=f"ct{c}") for c in range(4)]
                qT = st["qkt"][(0, pair)]
                kT = st["qkt"][(1, pair)]
                vst = st["vst"]
                ctxT = st["ctxT"]
                for sub in range(2):
                    h = 2 * pair + sub
                    hr = slice(sub * 64, sub * 64 + 64)
                    for scol in range(NCOL):
                        so = scol * 512
                        pex = []
                        for ti in range(NSC):
                            pp = ps.tile([128, 512], F32, tag="pp", bufs=2)
                            nc.tensor.matmul(
                                pp[:], kT[hr, ti * 128:(ti + 1) * 128],
                                qT[hr, so:so + 512], start=True, stop=True)
                            pe = pl.tile([128, 512], BF16, tag=f"pex{ti}",
                                         bufs=1, name="pe")
                            nc.scalar.activation(pe[:], pp[:], AF.Exp,
                                                 scale=0.125)
                            pex.append(pe)
                        pc = ps.tile([65, 512], F32, tag="pc", bufs=2)
                        for ti in range(NSC):
                            nc.tensor.matmul(
                                pc[:], vst[ti][:, h * 65:h * 65 + 65],
                                pex[ti][:], start=(ti == 0),
                                stop=(ti == NSC - 1))
                        zr = pl.tile([64, 512], F32R, tag="bcs", bufs=2,
                                     name="zr")
                        nc.vector.tensor_copy(zr[0:1, :], pc[64:65, :])
                        pb = ps.tile([64, 512], F32, tag="pp", bufs=2)
                        nc.tensor.matmul(pb[:], t_cones[0:1, 0:64], zr[0:1, :],
                                         start=True, stop=True)
                        bcs = pl.tile([64, 512], F32, tag="bcs", bufs=2,
                                      name="bcs")
                        nc.vector.reciprocal_approx_fast(out=bcs[:], in_=pb[:])
                        nc.vector.tensor_tensor(
                            ctxT[pair][hr, so:so + 512], pc[0:64, :],
                            bcs[:], ALU.mult)


            def emit_tail(it):
                """Wo + residual + LN1 + transpose into hT (+ h spill)."""
                st = state[it]
                ctxT = st["ctxT"]
                st_sum = pl.tile([128, NSC], F32, tag="st_sum", bufs=2)
                st_sq = pl.tile([128, NSC], F32, tag="st_sq", bufs=2)
                rr = []
                for sc in range(NSC):
                    xpt = pl.tile([128, 512], F32, tag="xpt", bufs=2)
                    nc.sync.dma_start(xpt[:], d_xp[it, sc])
                    pw = ps.tile([128, 512], F32, tag="pc", bufs=2)
                    for c in range(4):
                        nc.tensor.matmul(
                            pw[:], ctxT[c][:, sc * 128:(sc + 1) * 128],
                            t_wo[c][:], start=(c == 0), stop=(c == 3))
                    r = pl.tile([128, 512], F32, tag=f"res{sc}", name="r")
                    nc.vector.tensor_tensor(r[:], pw[:], xpt[:], ALU.add)
                    nc.vector.reduce_sum(st_sum[:, sc:sc + 1], r[:], axis=AX.X)
                    sq = pl.tile([128, 512], BF16, tag="sqs", bufs=2, name="sq")
                    nc.scalar.activation(sq[:], r[:], AF.Square,
                                         accum_out=st_sq[:, sc:sc + 1])
                    rr.append(r)
                mean8 = pl.tile([128, NSC], F32, tag="mean8", bufs=2)
                inv8 = pl.tile([128, NSC], F32, tag="inv8", bufs=2)
                msq = pl.tile([128, NSC], F32, tag="msq", bufs=2)
                nc.vector.tensor_scalar_mul(mean8[:], st_sum[:], 1.0 / D)
                nc.vector.tensor_scalar_mul(inv8[:], st_sq[:], 1.0 / D)
                nc.vector.tensor_tensor(msq[:], mean8[:], mean8[:], ALU.mult)
                nc.vector.tensor_tensor(inv8[:], inv8[:], msq[:], ALU.subtract)
                nc.scalar.activation(inv8[:], inv8[:], AF.Sqrt, bias=t_eps[:])
                nc.vector.reciprocal(inv8[:], inv8[:])
                for sc in range(NSC):
                    ht_ = pl.tile([128, 512], F32, tag="hst", bufs=2, name="h_")
                    nc.vector.tensor_scalar(
                        ht_[:], rr[sc][:], mean8[:, sc:sc + 1],
                        inv8[:, sc:sc + 1], ALU.subtract, ALU.mult)
                    nc.vector.tensor_tensor(ht_[:], ht_[:], t_gb[G1][:], ALU.mult)
                    nc.vector.tensor_tensor(ht_[:], ht_[:], t_gb[B1][:], ALU.add)
                    nc.sync.dma_start(h_dram[it][sc][:], ht_[:])
                    for dc in range(NDC):
                        pt = ps.tile([128, 128], F32, tag="pp", bufs=2)
                        nc.tensor.transpose(pt[:], ht_[:, dc * 128:(dc + 1) * 128],
                                            t_ident[:])
                        nc.scalar.copy(
                            hT[it][dc][:, 4 + sc * 128: 4 + (sc + 1) * 128],
                            pt[:])
                for dc in range(NDC):
                    nc.sync.dma_start(hT[it][dc][:, 0:4], d_czero[:, 0:4])
                    nc.sync.dma_start(hT[it][dc][:, S + 4:S + 8],
                                      d_czero[:, 4:8])

            o2 = [[None] * NSC for _ in range(NIT)]

            def emit_conv_chunk(it, cdc):
                w2t = pl.tile([128, KS * 512], BF16, tag="w2t", bufs=2,
                              name="w2t")
                nc.sync.dma_start(w2t[:], d_w2[cdc])
                w1t = []
                for dc in range(NDC):
                    t = pl.tile([128, KS * 128], BF16, tag=f"w1t{dc}", bufs=2,
                                name="w1t")
                    nc.sync.dma_start(t[:], d_w1[cdc, dc])
                    w1t.append(t)
                c1t = pl.tile([128, S + 8], BF16, tag="c1t", bufs=2, name="c1t")
                nc.sync.dma_start(c1t[:, 0:4], d_czero[:, 0:4])
                nc.sync.dma_start(c1t[:, S + 4:S + 8], d_czero[:, 4:8])
                for scol in range(NCOL):
                    pc1 = ps.tile([128, 512], F32, tag="c1p", bufs=2)
                    idx = 0
                    for k in range(KS):
                        for dc in range(NDC):
                            nc.tensor.matmul(
                                pc1[:], w1t[dc][:, k * 128:(k + 1) * 128],
                                hT[it][dc][:, scol * 512 + k:
                                           scol * 512 + k + 512],
                                start=(idx == 0), stop=(idx == 35))
                            idx += 1
                    nc.scalar.activation(
                        c1t[:, 4 + scol * 512: 4 + (scol + 1) * 512],
                        pc1[:], AF.Relu, bias=t_bc1s[:, cdc:cdc + 1])
                for sc in range(NSC):
                    pc2 = ps.tile([128, 512], F32, tag="c2p", bufs=2)
                    for k in range(KS):
                        nc.tensor.matmul(
                            pc2[:], c1t[:, sc * 128 + k: sc * 128 + k + 128],
                            w2t[:, k * 512:(k + 1) * 512],
                            start=(k == 0), stop=(k == KS - 1))
                    if cdc == 0:
                        t = pl.tile([128, 512], F32, tag=f"o2_{sc}",
                                    name=f"o2_{sc}")
                        o2[it][sc] = t
                        nc.vector.tensor_copy(t[:], pc2[:])
                    else:
                        nc.vector.tensor_tensor(o2[it][sc][:], pc2[:],
                                                o2[it][sc][:], ALU.add)

            def emit_ln2(it):
                st_sum = pl.tile([128, NSC], F32, tag="st_sum", bufs=2)
                st_sq = pl.tile([128, NSC], F32, tag="st_sq", bufs=2)
                rr = []
                for sc in range(NSC):
                    t1 = pl.tile([128, 512], F32, tag="hst", bufs=2)
                    nc.vector.tensor_tensor(t1[:], o2[it][sc][:], t_gb[BC2][:],
                                            ALU.add)
                    nc.scalar.activation(t1[:], t1[:], AF.Relu)
                    hrl = pl.tile([128, 512], F32, tag="xpt", bufs=2)
                    nc.sync.dma_start(hrl[:], h_dram[it][sc][:])
                    r = pl.tile([128, 512], F32, tag=f"res{sc}", name="r2")
                    nc.vector.tensor_tensor(r[:], t1[:], hrl[:], ALU.add)
                    nc.vector.reduce_sum(st_sum[:, sc:sc + 1], r[:], axis=AX.X)
                    sq = pl.tile([128, 512], BF16, tag="sqs", bufs=2, name="sq2")
                    nc.scalar.activation(sq[:], r[:], AF.Square,
                                         accum_out=st_sq[:, sc:sc + 1])
                    rr.append(r)
                mean8 = pl.tile([128, NSC], F32, tag="mean8", bufs=2)
                inv8 = pl.tile([128, NSC], F32, tag="inv8", bufs=2)
                msq = pl.tile([128, NSC], F32, tag="msq", bufs=2)
                nc.vector.tensor_scalar_mul(mean8[:], st_sum[:], 1.0 / D)
                nc.vector.tensor_scalar_mul(inv8[:], st_sq[:], 1.0 / D)
                nc.vector.tensor_tensor(msq[:], mean8[:], mean8[:], ALU.mult)
                nc.vector.tensor_tensor(inv8[:], inv8[:], msq[:], ALU.subtract)
                nc.scalar.activation(inv8[:], inv8[:], AF.Sqrt, bias=t_eps[:])
                nc.vector.reciprocal(inv8[:], inv8[:])
                for sc in range(NSC):
                    yt = pl.tile([128, 512], F32, tag="hst", bufs=2)
                    nc.vector.tensor_scalar(
                        yt[:], rr[sc][:], mean8[:, sc:sc + 1],
                        inv8[:, sc:sc + 1], ALU.subtract, ALU.mult)
                    nc.vector.tensor_tensor(yt[:], yt[:], t_gb[G2][:], ALU.mult)
                    nc.vector.tensor_tensor(yt[:], yt[:], t_gb[B2][:], ALU.add)
                    nc.sync.dma_start(d_y[it, sc], yt[:])

            # ================= emission order =================
            emit_x(0)
            emit_v(0)
            for pair in range(4):
                emit_qk(0, pair)
            emit_x(1)
            for pair in range(4):
                emit_heads_pair(0, pair)
                emit_qk(1, pair)
            emit_v(1)
            emit_tail(0)
            for cdc in range(NCD):
                emit_conv_chunk(0, cdc)
                if cdc < 4:
                    emit_heads_pair(1, cdc)
                elif cdc == 7:
                    emit_tail(1)
            emit_ln2(0)
            for cdc in range(NCD):
                emit_conv_chunk(1, cdc)
            emit_ln2(1)

    nc.compile()
    _BUILT = nc
    return nc


def _prep_host(inputs):
    import ml_dtypes
    bf16 = ml_dtypes.bfloat16
    x = np.asarray(inputs["x"], np.float32)
    Wq = np.asarray(inputs["Wq"], np.float32)
    bq = np.asarray(inputs["bq"], np.float32)
    Wk = np.asarray(inputs["Wk"], np.float32)
    bk = np.asarray(inputs["bk"], np.float32)
    Wv = np.asarray(inputs["Wv"], np.float32)
    bv = np.asarray(inputs["bv"], np.float32)
    Wo = np.asarray(inputs["Wo"], np.float32)
    bo = np.asarray(inputs["bo"], np.float32)
    g1 = np.asarray(inputs["g1"], np.float32)
    b1 = np.asarray(inputs["b1"], np.float32)
    g2 = np.asarray(inputs["g2"], np.float32)
    b2 = np.asarray(inputs["b2"], np.float32)
    Wc1 = np.asarray(inputs["Wc1"], np.float32)
    bc1 = np.asarray(inputs["bc1"], np.float32)
    Wc2 = np.asarray(inputs["Wc2"], np.float32)
    bc2 = np.asarray(inputs["bc2"], np.float32)

    xT = np.ascontiguousarray(x.transpose(0, 2, 1).reshape(B, NDC, 128, S))
    xp = np.ascontiguousarray((x + bo[None, None, :]).reshape(B, NSC, 128, D))

    wqk = np.zeros((2, 4, 128, 512), np.float32)
    for proj, W in ((0, Wq), (1, Wk)):
        for pair in range(4):
            blk = np.concatenate([W[2 * pair], W[2 * pair + 1]], axis=1)
            wqk[proj, pair] = blk.reshape(NDC, 128, 128).transpose(1, 0, 2) \
                                 .reshape(128, 512)
    bqk = np.zeros((128, 8), np.float32)
    for proj, b in ((0, bq), (1, bk)):
        for pair in range(4):
            bqk[:, proj * 4 + pair] = np.concatenate(
                [b[2 * pair], b[2 * pair + 1]])

    wv = np.zeros((NDC, 128, 520), np.float32)
    bvrow = np.zeros((128, 520), np.float32)
    for h in range(H):
        wv[:, :, h * 65:h * 65 + 64] = Wv[h].reshape(NDC, 128, 64)
        bvrow[0, h * 65:h * 65 + 64] = bv[h]
        bvrow[0, h * 65 + 64] = 1.0

    wo = np.ascontiguousarray(Wo.reshape(4, 128, 512))

    w1 = np.ascontiguousarray(
        Wc1.reshape(NCD, 128, NDC, 128, KS).transpose(0, 2, 3, 4, 1)
           .reshape(NCD, NDC, 128, KS * 128)).astype(bf16)
    w2 = np.ascontiguousarray(
        Wc2.reshape(D, NCD, 128, KS).transpose(1, 2, 3, 0)
           .reshape(NCD, 128, KS * 512)).astype(bf16)
    bc1s = np.ascontiguousarray(bc1.reshape(NCD, 128).T)

    gb = np.stack([np.tile(v[None, :], (128, 1))
                   for v in (g1, b1, g2, b2, bc2)]).astype(np.float32)
    cones = np.ones((128, 128), np.float32)
    czero = np.zeros((128, 8), bf16)

    shared = dict(wqk=wqk, bqk=bqk, wv=wv, bvrow=bvrow, wo=wo,
                  w1=w1, w2=w2, bc1s=bc1s, gb=gb, cones=cones, czero=czero)
    in_maps = []
    for c in range(NCORES):
        m = dict(shared)
        m["xT"] = np.ascontiguousarray(xT[c * NIT:(c + 1) * NIT])
        m["xp"] = np.ascontiguousarray(xp[c * NIT:(c + 1) * NIT])
        in_maps.append(m)
    return in_maps


def run(inputs, trace=False, **trace_kwargs):
    nc = _build()
    from concourse.bass_utils import run_bass_kernel_spmd
    in_maps = _prep_host(inputs)
    res = run_bass_kernel_spmd(nc, in_maps, core_ids=list(range(NCORES)),
                               trace=trace, **trace_kwargs)
    y = np.concatenate([res.results[c]["y"].reshape(NIT, S, D)
                        for c in range(NCORES)], axis=0)
    return y, res


def kernel(**inputs):
    y, _ = run(inputs, trace=False)
    return y


# revision 18
# speedup vs baseline: 1.1729x; 1.0057x over previous
"""Trainium2 Bass kernel for the FFT-block (attention + conv FFN) problem.

Sharding: data-parallel over batch. B=16 items across 8 cores -> 2 items/core.
Each core runs the full block for its items; no collectives.

Per item:
  - attention via scores^T = K Q^T (softmax sums land on the partition axis and
    are folded into the ctx matmul through a ones-column appended to V); the
    per-head 1/Z normalization is broadcast across partitions with a K=1 PE
    matmul.  Attention matmuls run in fp32r (tf32-like, fp32 accumulate);
    softmax weights and V are bf16.
  - convs are 9 shifted matmuls over transposed activations hT [D, S_pad] in
    bf16 (weights+activations), fp32 PSUM accumulation and fp32 o2 accumulator.
  - emission order software-pipelines item1's attention into item0's conv
    stream so the PE never drains (HAM stays at K=8/8).
"""
import sys, types
import numpy as np

B, S, D = 16, 1024, 512
H, DK = 8, 64
CD, KS = 2048, 9
EPS = 1e-5
NCORES = 8
NIT = B // NCORES
NDC = D // 128             # 4 d-chunks
NSC = S // 128             # 8 s-chunks
NCOL = S // 512            # 2 s-cols
NCD = CD // 128            # 16 cd-chunks


def _install_ntff_hook():
    try:
        from antenv.axon_hooks import get_axon_ntff_profile_hook  # noqa
        return
    except ImportError:
        pass
    try:
        from trn_agent_boot.trn_boot import _ntff_profile_via_ctypes
        mod = types.ModuleType('antenv.axon_hooks')
        hook = _ntff_profile_via_ctypes('/opt/axon/libaxon_pjrt.so')
        mod.get_axon_ntff_profile_hook = lambda: hook
        sys.modules['antenv.axon_hooks'] = mod
    except Exception:
        pass


_BUILT = None


def _build():
    global _BUILT
    if _BUILT is not None:
        return _BUILT
    _install_ntff_hook()
    import concourse.bacc as bacc
    import concourse.mybir as mybir
    from concourse import tile
    from concourse.masks import make_identity
    from contextlib import ExitStack

    F32 = mybir.dt.float32
    F32R = mybir.dt.float32r
    BF16 = mybir.dt.bfloat16
    AF = mybir.ActivationFunctionType
    ALU = mybir.AluOpType
    AX = mybir.AxisListType

    nc = bacc.Bacc("TRN2", target_bir_lowering=False, debug=False,
                   num_devices=NCORES)

    # ---- DRAM I/O (per core) ----
    d_xT = nc.dram_tensor("xT", [NIT, NDC, 128, S], F32, kind="ExternalInput")
    d_xp = nc.dram_tensor("xp", [NIT, NSC, 128, D], F32, kind="ExternalInput")
    d_wqk = nc.dram_tensor("wqk", [2, 4, 128, 512], F32, kind="ExternalInput")
    d_bqk = nc.dram_tensor("bqk", [128, 8], F32, kind="ExternalInput")
    d_wv = nc.dram_tensor("wv", [NDC, 128, 520], F32, kind="ExternalInput")
    d_bvrow = nc.dram_tensor("bvrow", [128, 520], F32, kind="ExternalInput")
    d_wo = nc.dram_tensor("wo", [4, 128, 512], F32, kind="ExternalInput")
    d_w1 = nc.dram_tensor("w1", [NCD, NDC, 128, KS * 128], BF16,
                          kind="ExternalInput")
    d_w2 = nc.dram_tensor("w2", [NCD, 128, KS * 512], BF16,
                          kind="ExternalInput")
    d_bc1s = nc.dram_tensor("bc1s", [128, NCD], F32, kind="ExternalInput")
    d_gb = nc.dram_tensor("gb", [5, 128, 512], F32, kind="ExternalInput")
    d_cones = nc.dram_tensor("cones", [128, 128], F32, kind="ExternalInput")
    d_czero = nc.dram_tensor("czero", [128, 8], BF16, kind="ExternalInput")
    d_y = nc.dram_tensor("y", [NIT, NSC, 128, D], F32, kind="ExternalOutput")

    G1, B1, G2, B2, BC2 = range(5)

    with tile.TileContext(nc) as tc:
        est = ExitStack()
        with est:
            cp = est.enter_context(tc.tile_pool(name="const", bufs=1))
            pl = est.enter_context(tc.tile_pool(name="work", bufs=1))
            ps = est.enter_context(tc.tile_pool(name="psum", bufs=1, space="PSUM"))
            dp = est.enter_context(tc.tile_pool(name="dramp", bufs=1, space="DRAM"))

            h_dram = [[dp.tile([128, D], F32, tag=f"hd{it}_{sc}",
                               name=f"hd{it}_{sc}")
                       for sc in range(NSC)] for it in range(NIT)]

            # ---- constants ----
            t_bqk = cp.tile([128, 8], F32, tag="bqk")
            nc.sync.dma_start(t_bqk[:], d_bqk[:])
            t_gb = []
            for i in range(5):
                t = cp.tile([128, 512], F32, tag=f"gb{i}", name=f"gb{i}")
                nc.sync.dma_start(t[:], d_gb[i])
                t_gb.append(t)
            t_bc1s = cp.tile([128, NCD], F32, tag="bc1s")
            nc.sync.dma_start(t_bc1s[:], d_bc1s[:])
            t_ident = cp.tile([128, 128], F32, tag="ident")
            make_identity(nc, t_ident[:])
            t_cones = cp.tile([128, 128], F32R, tag="cones")
            nc.sync.dma_start(t_cones[:], d_cones[:].bitcast(F32R))
            t_czero = cp.tile([128, 8], BF16, tag="czero")
            nc.sync.dma_start(t_czero[:], d_czero[:])
            t_eps = cp.tile([128, 1], F32, tag="eps")
            nc.vector.memset(t_eps[:], EPS)
            t_wv = []
            for dc in range(NDC):
                t = cp.tile([128, 520], F32R, tag=f"wv{dc}", name=f"wv{dc}")
                nc.sync.dma_start(t[:], d_wv[dc].bitcast(F32R))
                t_wv.append(t)
            t_bvrow = cp.tile([128, 520], F32R, tag="bvrow")
            nc.sync.dma_start(t_bvrow[:], d_bvrow[:].bitcast(F32R))
            t_wo = []
            for c in range(4):
                t = cp.tile([128, 512], F32R, tag=f"wo{c}", name=f"wo{c}")
                nc.sync.dma_start(t[:], d_wo[c].bitcast(F32R))
                t_wo.append(t)

            # persistent hT tiles (bf16, padded s)
            hT = [[pl.tile([128, S + 8], BF16, tag=f"ht{it}_{dc}",
                           name=f"ht{it}_{dc}")
                   for dc in range(NDC)] for it in range(NIT)]

            state = [dict() for _ in range(NIT)]

            # ================= emit helpers =================
            def emit_x(it):
                st = state[it]
                xt = []
                for dc in range(NDC):
                    t = pl.tile([128, S], F32R, tag=f"xt{dc}", name=f"xt{dc}")
                    nc.sync.dma_start(t[:], d_xT[it, dc].bitcast(F32R))
                    xt.append(t)
                st["xt"] = xt
                st["qkt"] = {}

            def emit_v(it):
                """V projection for one item (dense PE block)."""
                st = state[it]
                xt = st["xt"]
                vst = []
                for tc_i in range(NSC):
                    vt = pl.tile([128, 520], BF16, tag=f"vst{tc_i}",
                                 name=f"vst{tc_i}")
                    for half in range(2):
                        colo = half * 260
                        pv = ps.tile([128, 260], F32, tag="pp", bufs=3)
                        for dc in range(NDC):
                            nc.tensor.matmul(
                                pv[:], xt[dc][:, tc_i * 128:(tc_i + 1) * 128],
                                t_wv[dc][:, colo:colo + 260],
                                start=(dc == 0), stop=False)
                        nc.tensor.matmul(
                            pv[:], t_cones[0:1, 0:128],
                            t_bvrow[0:1, colo:colo + 260],
                            start=False, stop=True)
                        nc.vector.tensor_copy(vt[:, colo:colo + 260], pv[:])
                    vst.append(vt)
                st["vst"] = vst

            def emit_qk(it, pair):
                st = state[it]
                xt = st["xt"]
                for proj in range(2):
                    wt = pl.tile([128, 512], F32R, tag=f"wqk{proj}",
                                 bufs=2, name="wt")
                    nc.sync.dma_start(wt[:], d_wqk[proj, pair].bitcast(F32R))
                    qt = pl.tile([128, S], BF16, tag=f"qk{proj}{pair}",
                                 name="qt")
                    for scol in range(NCOL):
                        pq = ps.tile([128, 512], F32, tag="pp", bufs=3)
                        for dc in range(NDC):
                            nc.tensor.matmul(
                                pq[:], wt[:, dc * 128:(dc + 1) * 128],
                                xt[dc][:, scol * 512:(scol + 1) * 512],
                                start=(dc == 0), stop=(dc == NDC - 1))
                        nc.vector.tensor_scalar_add(
                            qt[:, scol * 512:(scol + 1) * 512], pq[:],
                            t_bqk[:, proj * 4 + pair:proj * 4 + pair + 1])
                    st["qkt"][(proj, pair)] = qt

            def emit_heads_pair(it, pair):
                st = state[it]
                if pair == 0:
                    st["ctxT"] = [pl.tile([128, S], F32R, tag=f"ct{c}",
                                          name=f"ct{c}") for c in range(4)]
                qT = st["qkt"][(0, pair)]
                kT = st["qkt"][(1, pair)]
                vst = st["vst"]
                ctxT = st["ctxT"]
                for sub in range(2):
                    h = 2 * pair + sub
                    hr = slice(sub * 64, sub * 64 + 64)
                    for scol in range(NCOL):
                        so = scol * 512
                        pex = []
                        for ti in range(NSC):
                            pp = ps.tile([128, 512], F32, tag="pp", bufs=3)
                            nc.tensor.matmul(
                                pp[:], kT[hr, ti * 128:(ti + 1) * 128],
                                qT[hr, so:so + 512], start=True, stop=True)
                            pe = pl.tile([128, 512], BF16, tag=f"pex{ti}",
                                         bufs=1, name="pe")
                            nc.scalar.activation(pe[:], pp[:], AF.Exp,
                                                 scale=0.125)
                            pex.append(pe)
                        pc = ps.tile([65, 512], F32, tag="pc", bufs=1)
                        for ti in range(NSC):
                            nc.tensor.matmul(
                                pc[:], vst[ti][:, h * 65:h * 65 + 65],
                                pex[ti][:], start=(ti == 0),
                                stop=(ti == NSC - 1))
                        zr = pl.tile([64, 512], F32R, tag="bcs", bufs=2,
                                     name="zr")
                        nc.vector.tensor_copy(zr[0:1, :], pc[64:65, :])
                        pb = ps.tile([64, 512], F32, tag="pp", bufs=3)
                        nc.tensor.matmul(pb[:], t_cones[0:1, 0:64], zr[0:1, :],
                                         start=True, stop=True)
                        bcs = pl.tile([64, 512], F32, tag="bcs", bufs=2,
                                      name="bcs")
                        nc.vector.reciprocal_approx_fast(out=bcs[:], in_=pb[:])
                        nc.vector.tensor_tensor(
                            ctxT[pair][hr, so:so + 512], pc[0:64, :],
                            bcs[:], ALU.mult)


            def emit_tail(it):
                """Wo + residual + LN1 + transpose into hT (+ h spill)."""
                st = state[it]
                ctxT = st["ctxT"]
                st_sum = pl.tile([128, NSC], F32, tag="st_sum", bufs=2)
                st_sq = pl.tile([128, NSC], F32, tag="st_sq", bufs=2)
                rr = []
                for sc in range(NSC):
                    xpt = pl.tile([128, 512], F32, tag="xpt", bufs=2)
                    nc.sync.dma_start(xpt[:], d_xp[it, sc])
                    pw = ps.tile([128, 512], F32, tag="pc", bufs=1)
                    for c in range(4):
                        nc.tensor.matmul(
                            pw[:], ctxT[c][:, sc * 128:(sc + 1) * 128],
                            t_wo[c][:], start=(c == 0), stop=(c == 3))
                    r = pl.tile([128, 512], F32, tag=f"res{sc}", name="r")
                    nc.vector.tensor_tensor(r[:], pw[:], xpt[:], ALU.add)
                    nc.vector.reduce_sum(st_sum[:, sc:sc + 1], r[:], axis=AX.X)
                    sq = pl.tile([128, 512], BF16, tag="sqs", bufs=2, name="sq")
                    nc.scalar.activation(sq[:], r[:], AF.Square,
                                         accum_out=st_sq[:, sc:sc + 1])
                    rr.append(r)
                mean8 = pl.tile([128, NSC], F32, tag="mean8", bufs=2)
                inv8 = pl.tile([128, NSC], F32, tag="inv8", bufs=2)
                msq = pl.tile([128, NSC], F32, tag="msq", bufs=2)
                nc.vector.tensor_scalar_mul(mean8[:], st_sum[:], 1.0 / D)
                nc.vector.tensor_scalar_mul(inv8[:], st_sq[:], 1.0 / D)
                nc.vector.tensor_tensor(msq[:], mean8[:], mean8[:], ALU.mult)
                nc.vector.tensor_tensor(inv8[:], inv8[:], msq[:], ALU.subtract)
                nc.scalar.activation(inv8[:], inv8[:], AF.Sqrt, bias=t_eps[:])
                nc.vector.reciprocal(inv8[:], inv8[:])
                for sc in range(NSC):
                    ht_ = pl.tile([128, 512], F32, tag="hst", bufs=2, name="h_")
                    nc.vector.tensor_scalar(
                        ht_[:], rr[sc][:], mean8[:, sc:sc + 1],
                        inv8[:, sc:sc + 1], ALU.subtract, ALU.mult)
                    nc.vector.tensor_tensor(ht_[:], ht_[:], t_gb[G1][:], ALU.mult)
                    nc.vector.tensor_tensor(ht_[:], ht_[:], t_gb[B1][:], ALU.add)
                    nc.sync.dma_start(h_dram[it][sc][:], ht_[:])
                    for dc in range(NDC):
                        pt = ps.tile([128, 128], F32, tag="pp", bufs=3)
                        nc.tensor.transpose(pt[:], ht_[:, dc * 128:(dc + 1) * 128],
                                            t_ident[:])
                        nc.scalar.copy(
                            hT[it][dc][:, 4 + sc * 128: 4 + (sc + 1) * 128],
                            pt[:])
                for dc in range(NDC):
                    nc.sync.dma_start(hT[it][dc][:, 0:4], d_czero[:, 0:4])
                    nc.sync.dma_start(hT[it][dc][:, S + 4:S + 8],
                                      d_czero[:, 4:8])

            o2 = [[None] * NSC for _ in range(NIT)]

            def emit_conv_chunk(it, cdc):
                w2t = pl.tile([128, KS * 512], BF16, tag="w2t", bufs=2,
                              name="w2t")
                nc.sync.dma_start(w2t[:], d_w2[cdc])
                w1t = []
                for dc in range(NDC):
                    t = pl.tile([128, KS * 128], BF16, tag=f"w1t{dc}", bufs=2,
                                name="w1t")
                    nc.sync.dma_start(t[:], d_w1[cdc, dc])
                    w1t.append(t)
                c1t = pl.tile([128, S + 8], BF16, tag="c1t", bufs=2, name="c1t")
                nc.sync.dma_start(c1t[:, 0:4], d_czero[:, 0:4])
                nc.sync.dma_start(c1t[:, S + 4:S + 8], d_czero[:, 4:8])
                for scol in range(NCOL):
                    pc1 = ps.tile([128, 512], F32, tag="c1p", bufs=2)
                    idx = 0
                    for k in range(KS):
                        for dc in range(NDC):
                            nc.tensor.matmul(
                                pc1[:], w1t[dc][:, k * 128:(k + 1) * 128],
                                hT[it][dc][:, scol * 512 + k:
                                           scol * 512 + k + 512],
                                start=(idx == 0), stop=(idx == 35))
                            idx += 1
                    nc.scalar.activation(
                        c1t[:, 4 + scol * 512: 4 + (scol + 1) * 512],
                        pc1[:], AF.Relu, bias=t_bc1s[:, cdc:cdc + 1])
                for sc in range(NSC):
                    pc2 = ps.tile([128, 512], F32, tag="c2p", bufs=2)
                    for k in range(KS):
                        nc.tensor.matmul(
                            pc2[:], c1t[:, sc * 128 + k: sc * 128 + k + 128],
                            w2t[:, k * 512:(k + 1) * 512],
                            start=(k == 0), stop=(k == KS - 1))
                    if cdc == 0:
                        t = pl.tile([128, 512], F32, tag=f"o2_{sc}",
                                    name=f"o2_{sc}")
                        o2[it][sc] = t
                        nc.vector.tensor_copy(t[:], pc2[:])
                    else:
                        nc.vector.tensor_tensor(o2[it][sc][:], pc2[:],
                                                o2[it][sc][:], ALU.add)

            def emit_ln2(it):
                st_sum = pl.tile([128, NSC], F32, tag="st_sum", bufs=2)
                st_sq = pl.tile([128, NSC], F32, tag="st_sq", bufs=2)
                rr = []
                for sc in range(NSC):
                    t1 = pl.tile([128, 512], F32, tag="hst", bufs=2)
                    nc.vector.tensor_tensor(t1[:], o2[it][sc][:], t_gb[BC2][:],
                                            ALU.add)
                    nc.scalar.activation(t1[:], t1[:], AF.Relu)
                    hrl = pl.tile([128, 512], F32, tag="xpt", bufs=2)
                    nc.sync.dma_start(hrl[:], h_dram[it][sc][:])
                    r = pl.tile([128, 512], F32, tag=f"res{sc}", name="r2")
                    nc.vector.tensor_tensor(r[:], t1[:], hrl[:], ALU.add)
                    nc.vector.reduce_sum(st_sum[:, sc:sc + 1], r[:], axis=AX.X)
                    sq = pl.tile([128, 512], BF16, tag="sqs", bufs=2, name="sq2")
                    nc.scalar.activation(sq[:], r[:], AF.Square,
                                         accum_out=st_sq[:, sc:sc + 1])
                    rr.append(r)
                mean8 = pl.tile([128, NSC], F32, tag="mean8", bufs=2)
                inv8 = pl.tile([128, NSC], F32, tag="inv8", bufs=2)
                msq = pl.tile([128, NSC], F32, tag="msq", bufs=2)
                nc.vector.tensor_scalar_mul(mean8[:], st_sum[:], 1.0 / D)
                nc.vector.tensor_scalar_mul(inv8[:], st_sq[:], 1.0 / D)
                nc.vector.tensor_tensor(msq[:], mean8[:], mean8[:], ALU.mult)
                nc.vector.tensor_tensor(inv8[:], inv8[:], msq[:], ALU.subtract)
                nc.scalar.activation(inv8[:], inv8[:], AF.Sqrt, bias=t_eps[:])
                nc.vector.reciprocal(inv8[:], inv8[:])
                for sc in range(NSC):
                    yt = pl.tile([128, 512], F32, tag="hst", bufs=2)
                    nc.vector.tensor_scalar(
                        yt[:], rr[sc][:], mean8[:, sc:sc + 1],
                        inv8[:, sc:sc + 1], ALU.subtract, ALU.mult)
                    nc.vector.tensor_tensor(yt[:], yt[:], t_gb[G2][:], ALU.mult)
                    nc.vector.tensor_tensor(yt[:], yt[:], t_gb[B2][:], ALU.add)
                    nc.sync.dma_start(d_y[it, sc], yt[:])

            # ================= emission order =================
            emit_x(0)
            emit_v(0)
            for pair in range(4):
                emit_qk(0, pair)
            emit_x(1)
            for pair in range(4):
                emit_heads_pair(0, pair)
                emit_qk(1, pair)
            emit_v(1)
            emit_tail(0)
            for cdc in range(NCD):
                emit_conv_chunk(0, cdc)
                if cdc < 4:
                    emit_heads_pair(1, cdc)
                elif cdc == 7:
                    emit_tail(1)
            emit_ln2(0)
            for cdc in range(NCD):
                emit_conv_chunk(1, cdc)
            emit_ln2(1)

    nc.compile()
    _BUILT = nc
    return nc


def _prep_host(inputs):
    import ml_dtypes
    bf16 = ml_dtypes.bfloat16
    x = np.asarray(inputs["x"], np.float32)
    Wq = np.asarray(inputs["Wq"], np.float32)
    bq = np.asarray(inputs["bq"], np.float32)
    Wk = np.asarray(inputs["Wk"], np.float32)
    bk = np.asarray(inputs["bk"], np.float32)
    Wv = np.asarray(inputs["Wv"], np.float32)
    bv = np.asarray(inputs["bv"], np.float32)
    Wo = np.asarray(inputs["Wo"], np.float32)
    bo = np.asarray(inputs["bo"], np.float32)
    g1 = np.asarray(inputs["g1"], np.float32)
    b1 = np.asarray(inputs["b1"], np.float32)
    g2 = np.asarray(inputs["g2"], np.float32)
    b2 = np.asarray(inputs["b2"], np.float32)
    Wc1 = np.asarray(inputs["Wc1"], np.float32)
    bc1 = np.asarray(inputs["bc1"], np.float32)
    Wc2 = np.asarray(inputs["Wc2"], np.float32)
    bc2 = np.asarray(inputs["bc2"], np.float32)

    xT = np.ascontiguousarray(x.transpose(0, 2, 1).reshape(B, NDC, 128, S))
    xp = np.ascontiguousarray((x + bo[None, None, :]).reshape(B, NSC, 128, D))

    wqk = np.zeros((2, 4, 128, 512), np.float32)
    for proj, W in ((0, Wq), (1, Wk)):
        for pair in range(4):
            blk = np.concatenate([W[2 * pair], W[2 * pair + 1]], axis=1)
            wqk[proj, pair] = blk.reshape(NDC, 128, 128).transpose(1, 0, 2) \
                                 .reshape(128, 512)
    bqk = np.zeros((128, 8), np.float32)
    for proj, b in ((0, bq), (1, bk)):
        for pair in range(4):
            bqk[:, proj * 4 + pair] = np.concatenate(
                [b[2 * pair], b[2 * pair + 1]])

    wv = np.zeros((NDC, 128, 520), np.float32)
    bvrow = np.zeros((128, 520), np.float32)
    for h in range(H):
        wv[:, :, h * 65:h * 65 + 64] = Wv[h].reshape(NDC, 128, 64)
        bvrow[0, h * 65:h * 65 + 64] = bv[h]
        bvrow[0, h * 65 + 64] = 1.0

    wo = np.ascontiguousarray(Wo.reshape(4, 128, 512))

    w1 = np.ascontiguousarray(
        Wc1.reshape(NCD, 128, NDC, 128, KS).transpose(0, 2, 3, 4, 1)
           .reshape(NCD, NDC, 128, KS * 128)).astype(bf16)
    w2 = np.ascontiguousarray(
        Wc2.reshape(D, NCD, 128, KS).transpose(1, 2, 3, 0)
           .reshape(NCD, 128, KS * 512)).astype(bf16)
    bc1s = np.ascontiguousarray(bc1.reshape(NCD, 128).T)

    gb = np.stack([np.tile(v[None, :], (128, 1))
                   for v in (g1, b1, g2, b2, bc2)]).astype(np.float32)
    cones = np.ones((128, 128), np.float32)
    czero = np.zeros((128, 8), bf16)

    shared = dict(wqk=wqk, bqk=bqk, wv=wv, bvrow=bvrow, wo=wo,
                  w1=w1, w2=w2, bc1s=bc1s, gb=gb, cones=cones, czero=czero)
    in_maps = []
    for c in range(NCORES):
        m = dict(shared)
        m["xT"] = np.ascontiguousarray(xT[c * NIT:(c + 1) * NIT])
        m["xp"] = np.ascontiguousarray(xp[c * NIT:(c + 1) * NIT])
        in_maps.append(m)
    return in_maps


def run(inputs, trace=False, **trace_kwargs):
    nc = _build()
    from concourse.bass_utils import run_bass_kernel_spmd
    in_maps = _prep_host(inputs)
    res = run_bass_kernel_spmd(nc, in_maps, core_ids=list(range(NCORES)),
                               trace=trace, **trace_kwargs)
    y = np.concatenate([res.results[c]["y"].reshape(NIT, S, D)
                        for c in range(NCORES)], axis=0)
    return y, res


def kernel(**inputs):
    y, _ = run(inputs, trace=False)
    return y
